# revision 1
# baseline (speedup 1.0000x reference)
"""CARAFE-downsample Trainium2 kernel (B=8, C=256, H=W=128, K=5, S=2, M=64).

Sharding: data-parallel over batch B across 8 NeuronCores (one sample per core).

Per-core pipeline, fused per image-quarter (32 rows) so DMA loads, TensorE
matmuls, softmax, scatter and output stores all overlap:
  1. SWDGE cast-DMA  x[b] fp32 DRAM -> fp16 SBUF, channel-major [128c, 4096]
  2. xbar DMA-transpose -> xT quarter tiles, pixel-major [128 iw, 32 rows, 256 c]
  3. compress 1x1 conv (C=256 -> M=64): 2-chunk accumulating matmuls; bias
     fused in PSUM->SBUF copy into zero-padded per-quarter ker1 [64, 34, 130]
     (1-row halo duplicated across quarter boundaries)
  4. encoder 3x3 stride-2 conv (M=64 -> 25): 9 accumulating matmuls with
     stride-2 gather APs on ker1
  5. PE-transpose logits [25, 128pix] -> [128pix, 25]; softmax on free dim
  6. per output-row-pair P: gpsimd local_scatter builds A^T[p, (ridx, iw)]
     (scattered softmax weights, constant index map), xbar DMA-transpose -> A
  7. out[c, p] = sum_q xT[q, c]^T A[q, p]: <=7 accumulating matmuls per
     (P, c-chunk); out-of-image taps skipped (zero padding semantics)
  8. PSUM -> SBUF staging -> 512 KB DMAs to DRAM (channel-major, fp32)
"""

import sys

if "/opt/trn_rl_repo" not in sys.path:
    sys.path.insert(0, "/opt/trn_rl_repo")

import numpy as np

import concourse.bacc as bacc
import concourse.tile as tile
from concourse import mybir
from concourse.bass_utils import run_bass_kernel_spmd

F32 = mybir.dt.float32
BF16 = mybir.dt.float16          # 2-byte compute dtype (fp16)
I16 = mybir.dt.int16

B, C, H, W = 8, 256, 128, 128
M = 64          # compressed channels
K = 5           # carafe kernel size
S = 2           # stride
KK = K * K      # 25
nH, nW = H // S, W // S          # 64, 64
NPIX = H * W                     # 16384
NOPIX = nH * nW                  # 4096
NPAIR = nH // 2                  # 32 output-row pairs
QCH = 7                          # input rows per pair band (4P-2 .. 4P+4)
APITCH = 144                     # free pitch of A chunks (non-mergeable, 32B-aligned)
NIDX = 26                        # local_scatter num_idxs (25 taps + pad)


def _exp_taps():
    """Constant per-partition scatter index map: idx[p, k] -> (ridx, iw) slot."""
    idx = np.full((128, NIDX), -1, dtype=np.int16)
    for p in range(128):
        doh, ow = p // nW, p % nW
        for k in range(KK):
            i, j = k // K, k % K
            iw = 2 * ow + j - 2
            if 0 <= iw < W:
                idx[p, k] = (2 * doh + i) * W + iw
    return idx


def _build_program(repeat=1):
    nc = bacc.Bacc("TRN2", target_bir_lowering=False, debug=False, num_devices=8)

    x_d = nc.dram_tensor("x", [C, NPIX], F32, kind="ExternalInput")
    w1_d = nc.dram_tensor("w1t", [128, 2, M], BF16, kind="ExternalInput")
    w2_d = nc.dram_tensor("w2t", [M, 9, KK], BF16, kind="ExternalInput")
    b1_d = nc.dram_tensor("b1c", [M, 1], F32, kind="ExternalInput")
    b2_d = nc.dram_tensor("b2c", [KK, 1], F32, kind="ExternalInput")
    id_d = nc.dram_tensor("ident", [KK, KK], BF16, kind="ExternalInput")
    ix_d = nc.dram_tensor("scidx", [128, NIDX], I16, kind="ExternalInput")
    o_d = nc.dram_tensor("out", [C, NOPIX], F32, kind="ExternalOutput")

    with tile.TileContext(nc) as tc:
        with (
            tc.tile_pool(name="const", bufs=1) as constp,
            tc.tile_pool(name="xq", bufs=2) as xqp,
            tc.tile_pool(name="xt", bufs=2) as xtp,
            tc.tile_pool(name="k1", bufs=2) as k1p,
            tc.tile_pool(name="sm", bufs=3) as smp,
            tc.tile_pool(name="ab", bufs=3) as abp,
            tc.tile_pool(name="ost", bufs=2) as ostp,
            tc.tile_pool(name="psA", bufs=5, space="PSUM") as psA,
            tc.tile_pool(name="psB", bufs=2, space="PSUM") as psB,
        ):
            # ---- constants ----
            w1sb = constp.tile([128, 2, M], BF16)
            nc.sync.dma_start(out=w1sb[:], in_=w1_d.ap())
            w2sb = constp.tile([M, 9, KK], BF16)
            nc.sync.dma_start(out=w2sb[:], in_=w2_d.ap())
            b1sb = constp.tile([M, 1], F32)
            nc.sync.dma_start(out=b1sb[:], in_=b1_d.ap())
            b2sb = constp.tile([KK, 1], F32)
            nc.sync.dma_start(out=b2sb[:], in_=b2_d.ap())
            idsb = constp.tile([KK, KK], BF16)
            nc.sync.dma_start(out=idsb[:], in_=id_d.ap())
            ixsb = constp.tile([128, NIDX], I16)
            nc.sync.dma_start(out=ixsb[:], in_=ix_d.ap())

            for _rep in range(repeat):
                # per-quarter tensors, kept alive across the quarter pipeline
                xTq = [None] * 4
                k1q = [None] * 4

                def ensure_k1(q):
                    if k1q[q] is None:
                        k1q[q] = k1p.tile([M, 34, W + 2], BF16, tag=f"k1_{q % 2}", name=f"k1t{q}")
                        # zero the column pads; top image pad for tile 0
                        nc.vector.memset(k1q[q][:, 0:33, 0], 0.0)
                        nc.vector.memset(k1q[q][:, 0:33, W + 1], 0.0)
                        if q == 0:
                            nc.vector.memset(k1q[0][:, 0, :], 0.0)
                    return k1q[q]

                for q in range(4):
                    # ---- load + transpose + compress conv for quarter q ----
                    # 33-row load (1-row lookahead) so each pair band of
                    # group q lives entirely in xT tiles q-1 / q
                    nrow = 33 if q < 3 else 32
                    xq0 = xqp.tile([128, 33 * 128], BF16, tag="xq0")
                    xq1 = xqp.tile([128, 33 * 128], BF16, tag="xq1")
                    nc.gpsimd.dma_start(
                        out=xq0[:, 0 : nrow * 128],
                        in_=x_d.ap()[0:128, q * 4096 : q * 4096 + nrow * 128],
                    )
                    nc.gpsimd.dma_start(
                        out=xq1[:, 0 : nrow * 128],
                        in_=x_d.ap()[128:256, q * 4096 : q * 4096 + nrow * 128],
                    )
                    xTq[q] = xtp.tile([128, 33, C], BF16, tag=f"xt_{q % 2}", name=f"xtt{q}")
                    # batched xbar transpose: out[iw, r, c] = in[c, r*128 + iw]
                    nc.sync.dma_start(
                        out=xTq[q][:, 0:nrow, 0:128],
                        in_=xq0[:, 0 : nrow * 128],
                        transpose=True,
                    )
                    nc.sync.dma_start(
                        out=xTq[q][:, 0:nrow, 128:256],
                        in_=xq1[:, 0 : nrow * 128],
                        transpose=True,
                    )
                    k1c = ensure_k1(q)
                    for blk in range(8):    # 512-pixel (4-image-row) blocks
                        ps1 = psA.tile([M, 512], F32, tag="mm")
                        nc.tensor.matmul(
                            ps1[:],
                            lhsT=w1sb[:, 0, :],
                            rhs=xq0[:, blk * 512 : (blk + 1) * 512],
                            start=True,
                            stop=False,
                        )
                        nc.tensor.matmul(
                            ps1[:],
                            lhsT=w1sb[:, 1, :],
                            rhs=xq1[:, blk * 512 : (blk + 1) * 512],
                            start=False,
                            stop=True,
                        )
                        # local padded rows l = blk*4+1 .. blk*4+4
                        nc.vector.tensor_scalar_add(
                            out=k1c[:, blk * 4 + 1 : blk * 4 + 5, 1 : 1 + W],
                            in0=ps1[:].rearrange("p (r w) -> p r w", r=4),
                            scalar1=b1sb[:],
                        )
                        if blk == 7 and q < 3:
                            # halo: image row 32q+31 duplicated as next tile's row 0
                            k1n = ensure_k1(q + 1)
                            nc.vector.tensor_scalar_add(
                                out=k1n[:, 0:1, 1 : 1 + W],
                                in0=ps1[:, 3 * 128 : 512].rearrange(
                                    "p (r w) -> p r w", r=1
                                ),
                                scalar1=b1sb[:],
                            )

                    # ---- encoder conv + softmax + scatter + weighted sum ----
                    osb0 = ostp.tile([128, 8 * 128], F32, tag="o0")
                    osb1 = ostp.tile([128, 8 * 128], F32, tag="o1")
                    kvq = k1c[:].rearrange(
                        "p (r s) (w t) -> p r s w t", s=2, t=2
                    )  # [64, 17, 2, 65, 2]
                    for sub in range(2):
                        cb = 2 * q + sub
                        ps2 = psA.tile([KK, 512], F32, tag="mm")
                        for tap in range(9):
                            dy, dx = tap // 3, tap % 3
                            r0 = 8 * sub + dy // 2
                            n0 = dx // 2
                            rhs = kvq[:, r0 : r0 + 8, dy & 1, n0 : n0 + nW, dx & 1]
                            nc.tensor.matmul(
                                ps2[:],
                                lhsT=w2sb[:, tap, :],
                                rhs=rhs,
                                start=(tap == 0),
                                stop=(tap == 8),
                            )
                        ker2b = smp.tile([KK, 512], BF16, tag="k2")
                        nc.vector.tensor_scalar_add(
                            out=ker2b[:], in0=ps2[:], scalar1=b2sb[:]
                        )
                        # softmax over the 25 taps, pixel-major
                        ecb = smp.tile([128, 4, KK], F32, tag="e")
                        scb = smp.tile([128, 4], F32, tag="s")
                        rcb = smp.tile([128, 4], F32, tag="r")
                        for pp in range(4):
                            psT = psB.tile([128, KK], BF16, tag="tr")
                            nc.tensor.transpose(
                                psT[:],
                                ker2b[:, pp * 128 : (pp + 1) * 128],
                                idsb[:],
                            )
                            nc.scalar.activation(
                                out=ecb[:, pp, :],
                                in_=psT[:],
                                func=mybir.ActivationFunctionType.Exp,
                            )
                        nc.vector.tensor_reduce(
                            out=scb[:],
                            in_=ecb[:],
                            axis=mybir.AxisListType.X,
                            op=mybir.AluOpType.add,
                        )
                        nc.vector.reciprocal(out=rcb[:], in_=scb[:])
                        for pp in range(4):
                            P = 4 * cb + pp
                            wn = smp.tile([128, NIDX], BF16, tag="wn")
                            nc.vector.tensor_scalar_mul(
                                out=wn[:, 0:KK],
                                in0=ecb[:, pp, :],
                                scalar1=rcb[:, pp : pp + 1],
                            )
                            nc.vector.memset(wn[:, KK:NIDX], 0.0)
                            at = smp.tile([128, QCH * 128], BF16, tag="at")
                            nc.gpsimd.local_scatter(
                                out_ap=at[:],
                                data_ap=wn[:],
                                idxs_ap=ixsb[:],
                                channels=128,
                                num_elems=QCH * 128,
                                num_idxs=NIDX,
                            )
                            amat = abp.tile([128, QCH, APITCH], BF16, tag="A")
                            nc.sync.dma_start(
                                out=amat[:, :, 0:128], in_=at[:], transpose=True
                            )
                            qqs = [
                                qq
                                for qq in range(QCH)
                                if 0 <= 4 * P - 2 + qq < H
                            ]
                            for cc in range(2):
                                psF = psA.tile([128, 128], F32, tag="mm")
                                for qi, qq in enumerate(qqs):
                                    r = 4 * P - 2 + qq
                                    g = P // 8
                                    tl = g - 1 if r < 32 * g else g
                                    nc.tensor.matmul(
                                        psF[:],
                                        lhsT=xTq[tl][
                                            :, r - 32 * tl, cc * 128 : (cc + 1) * 128
                                        ],
                                        rhs=amat[:, qq, 0:128],
                                        start=(qi == 0),
                                        stop=(qi == len(qqs) - 1),
                                    )
                                osb = osb0 if cc == 0 else osb1
                                pos = 4 * sub + pp
                                if pos % 2 == 0:
                                    nc.scalar.copy(
                                        out=osb[:, pos * 128 : (pos + 1) * 128],
                                        in_=psF[:],
                                    )
                                else:
                                    nc.vector.tensor_copy(
                                        out=osb[:, pos * 128 : (pos + 1) * 128],
                                        in_=psF[:],
                                    )
                    nc.sync.dma_start(
                        out=o_d.ap()[0:128, q * 1024 : (q + 1) * 1024], in_=osb0[:]
                    )
                    nc.sync.dma_start(
                        out=o_d.ap()[128:256, q * 1024 : (q + 1) * 1024], in_=osb1[:]
                    )

    nc.compile()
    return nc


_NC = None


def _get_nc():
    global _NC
    if _NC is None:
        _NC = _build_program()
    return _NC


def _host_inputs(w1, b1, w2, b2):
    """Precompute constant / rearranged weight tensors (host-side, numpy)."""
    bf = np.float16
    w1m = np.asarray(w1, np.float32).reshape(M, C)            # [m, c]
    w1t = np.transpose(w1m.reshape(M, 2, 128), (2, 1, 0)).astype(bf)  # [cp, chunk, m]
    w1t = np.ascontiguousarray(w1t)
    w2m = np.asarray(w2, np.float32).reshape(KK, M, 9)        # [k, m, tap]
    w2t = np.ascontiguousarray(np.transpose(w2m, (1, 2, 0))).astype(bf)  # [m, tap, k]
    b1c = np.asarray(b1, np.float32).reshape(M, 1).copy()
    b2c = np.asarray(b2, np.float32).reshape(KK, 1).copy()
    ident = np.eye(KK, dtype=bf)
    scidx = _exp_taps()
    return {
        "w1t": w1t,
        "w2t": w2t,
        "b1c": b1c,
        "b2c": b2c,
        "ident": ident,
        "scidx": scidx,
    }


def kernel(x, w1, b1, w2, b2):
    x = np.asarray(x, np.float32)
    consts = _host_inputs(w1, b1, w2, b2)
    nc = _get_nc()
    in_maps = []
    for b in range(B):
        m = {"x": np.ascontiguousarray(x[b].reshape(C, NPIX))}
        m.update(consts)
        in_maps.append(m)
    res = run_bass_kernel_spmd(nc, in_maps, core_ids=list(range(B)))
    out = np.stack([res.results[i]["out"] for i in range(B)], axis=0)
    return out.reshape(B, C, nH, nW)



# revision 32
# speedup vs baseline: 42.1137x; 42.1137x over previous
"""CARAFE-downsample Trainium2 kernel (B=8, C=256, H=W=128, K=5, S=2, M=64).

Sharding: data-parallel over batch B across 8 NeuronCores (one sample per core).

v3: scheduling-oriented rewrite of the baseline (sim cost 97.4us vs 173us).
  * compress(1x1) + encoder(3x3,s2) convs fused into ONE composed conv:
    ker2 = (w2 o w1) * x  (host-precomputed weff[25,C,9], exact; per-pixel
    bias field beff handles border taps exactly for any b1/b2; dx=0 taps'
    bogus ow=0 column cancelled by negated-weight corrective matmuls)
  * per-engine queues specialized to avoid head-of-line blocking, with
    just-in-time staging (load/transpose of quarter q+1/q+2 emitted as
    quarter q computes):
      Pool: x cast-loads (fp32->fp16), local_scatter;
      SP:   input DMA-transposes, bf16 output stores, consts;
      Act:  softmax exp, half the PSUM->SBUF copies;
      DVE:  bias add, softmax reduce/recip/mul, other half of copies;
      PE:   composed conv, logit transposes, A-matrix transposes
            (via identity; Act-queue DMA-transpose is broken on HW),
            weighted-sum matmuls
  * weighted-sum matmuls for quarter q run one quarter late (skew) so the
    softmax->scatter->PE-transpose pipeline has a full quarter of slack
  * PSUM: 2 banks conv logits, 2 banks logit-transpose staging, 2 banks
    A-matrix staging, 2 banks weighted-sum accumulators [128,512]
"""

import sys

if "/opt/trn_rl_repo" not in sys.path:
    sys.path.insert(0, "/opt/trn_rl_repo")

import numpy as np

import concourse.bacc as bacc
import concourse.tile as tile
from concourse import mybir
from concourse.bass_utils import run_bass_kernel_spmd

F32 = mybir.dt.float32
BF16 = mybir.dt.float16          # 2-byte compute dtype (fp16)
I16 = mybir.dt.int16

B, C, H, W = 8, 256, 128, 128
M = 64          # compressed channels
K = 5           # carafe kernel size
S = 2           # stride
KK = K * K      # 25
nH, nW = H // S, W // S          # 64, 64
NPIX = H * W                     # 16384
NOPIX = nH * nW                  # 4096
NPAIR = nH // 2                  # 32 output-row pairs
QCH = 7                          # input rows per pair band (4P-2 .. 4P+4)
APITCH = 144                     # free pitch of A chunks (non-mergeable, 32B-aligned)
NIDX = 26                        # local_scatter num_idxs (25 taps + pad)


def _exp_taps():
    """Constant per-partition scatter index map: idx[p, k] -> (ridx, iw) slot."""
    idx = np.full((128, NIDX), -1, dtype=np.int16)
    for p in range(128):
        doh, ow = p // nW, p % nW
        for k in range(KK):
            i, j = k // K, k % K
            iw = 2 * ow + j - 2
            if 0 <= iw < W:
                idx[p, k] = (2 * doh + i) * W + iw
    return idx


def _build_program(repeat=1):
    nc = bacc.Bacc("TRN2", target_bir_lowering=False, debug=False, num_devices=8)

    x_d = nc.dram_tensor("x", [C, NPIX], F32, kind="ExternalInput")
    weff_d = nc.dram_tensor("weff", [128, 2, 9, KK], BF16, kind="ExternalInput")
    wneg_d = nc.dram_tensor("wneff", [128, 2, 3, KK], BF16, kind="ExternalInput")
    beff_d = nc.dram_tensor("beff", [KK, NOPIX], F32, kind="ExternalInput")
    id_d = nc.dram_tensor("ident", [KK, KK], BF16, kind="ExternalInput")
    id128_d = nc.dram_tensor("ident128", [128, 128], BF16, kind="ExternalInput")
    ix_d = nc.dram_tensor("scidx", [128, NIDX], I16, kind="ExternalInput")
    o_d = nc.dram_tensor("out", [C, NOPIX], BF16, kind="ExternalOutput")

    with tile.TileContext(nc) as tc:
        with (
            tc.tile_pool(name="const", bufs=1) as constp,
            tc.tile_pool(name="xq", bufs=3) as xqp,
            tc.tile_pool(name="xt", bufs=1) as xtp,
            tc.tile_pool(name="sm", bufs=2) as smp,
            tc.tile_pool(name="at", bufs=4) as atp,
            tc.tile_pool(name="ab", bufs=12) as abp,
            tc.tile_pool(name="ost", bufs=2) as ostp,
            tc.tile_pool(name="psE", bufs=2, space="PSUM") as psE,
            tc.tile_pool(name="psT", bufs=2, space="PSUM") as psT,
            tc.tile_pool(name="psF", bufs=2, space="PSUM") as psF,
            tc.tile_pool(name="psA", bufs=2, space="PSUM") as psA,
        ):
            # ---- constants ----
            wfsb = constp.tile([128, 2, 9, KK], BF16)
            nc.sync.dma_start(out=wfsb[:], in_=weff_d.ap())
            wnsb = constp.tile([128, 2, 3, KK], BF16)
            nc.sync.dma_start(out=wnsb[:], in_=wneg_d.ap())
            bfsb = constp.tile([KK, NOPIX], F32)
            nc.sync.dma_start(out=bfsb[:], in_=beff_d.ap())
            idsb = constp.tile([KK, KK], BF16)
            nc.sync.dma_start(out=idsb[:], in_=id_d.ap())
            id128 = constp.tile([128, 128], BF16)
            nc.sync.dma_start(out=id128[:], in_=id128_d.ap())
            ixsb = constp.tile([128, NIDX], I16)
            nc.sync.dma_start(out=ixsb[:], in_=ix_d.ap())

            for _rep in range(repeat):
                xqs = [[None, None] for _ in range(4)]   # channel-major windows
                xTq = [None] * 4                         # pixel-major quarters
                amats = [None] * NPAIR
                osbs = [None] * 4

                # quarter q channel-major window: contiguous [128, 35*128],
                # holding global rows r0(q) .. r0+nrow-1 at indices 0..nrow-1.
                # q >= 1 starts 2 rows early so conv taps never index row < 0.
                def q_window(q):
                    r0 = 0 if q == 0 else 32 * q - 2
                    nrow = {0: 33, 1: 35, 2: 35, 3: 34}[q]
                    return r0, nrow

                def emit_load(q):
                    r0, nrow = q_window(q)
                    for h in range(2):
                        xq = xqp.tile(
                            [128, 35 * 128], BF16, tag=f"xq{h}", name=f"xq{q}_{h}"
                        )
                        nc.gpsimd.dma_start(
                            out=xq[:, 0 : nrow * 128],
                            in_=x_d.ap()[
                                h * 128 : (h + 1) * 128,
                                r0 * 128 : (r0 + nrow) * 128,
                            ],
                        )
                        xqs[q][h] = xq

                def emit_transpose(q):
                    # xT[q]: pixel-major rows 32q .. 32q+32 (q=3: ..127)
                    nrow = 33 if q < 3 else 32
                    r0, _ = q_window(q)
                    skip = 32 * q - r0
                    xT = xtp.tile([128, 33, C], BF16, tag=f"xt{q}", name=f"xt{q}")
                    for h in range(2):
                        nc.sync.dma_start(
                            out=xT[:, 0:nrow, h * 128 : (h + 1) * 128],
                            in_=xqs[q][h][
                                :, skip * 128 : (skip + nrow) * 128
                            ],
                            transpose=True,
                        )
                    xTq[q] = xT

                # just-in-time staging: only what's needed soon goes on the
                # bus early; the rest is emitted as the loop progresses
                emit_load(0)
                emit_load(1)
                emit_transpose(0)

                def emit_apath(q):
                    r0, _ = q_window(q)
                    q_ats = pend_ats
                    for sub in range(2):
                        oh0 = 16 * q + 8 * sub       # first output row of sub
                        ps2 = psE.tile([KK, 512], F32, tag="e")
                        # composed conv: 18 full matmuls + up to 6 corrective
                        # matmuls (negated weights) cancelling the dx=0 taps'
                        # bogus ow=0 column (reads previous row's last pixel).
                        tap_order = [(1, 1), (1, 0), (1, 2),
                                     (0, 0), (0, 1), (0, 2),
                                     (2, 0), (2, 1), (2, 2)]
                        top = oh0 == 0
                        for mi in range(18):
                            cc = mi % 2
                            dy, dx = tap_order[mi // 2]
                            ti = dy * 3 + dx
                            row_base = 2 * oh0 + dy - 1 - r0  # window row idx
                            l_lo = 1 if (top and (dy == 0 or (dy, dx) == (1, 0))) else 0
                            l_cnt = 8 - l_lo
                            base = (row_base + 2 * l_lo) * 128 + dx - 1
                            v = xqs[q][cc][
                                :, base : base + (2 * l_cnt - 1) * 128
                            ].rearrange("p (l w) -> p l w", w=128)
                            mm_rhs = v[:, 0 : 2 * l_cnt - 1 : 2, 0:127:2]
                            nc.tensor.matmul(
                                ps2[:, l_lo * nW : 512],
                                lhsT=wfsb[:, cc, ti, :],
                                rhs=mm_rhs,
                                start=(mi == 0),
                                stop=False,
                            )
                            if top and (dy, dx) == (1, 0):
                                # l=0 row of the (1,0) tap: ow>=1 only
                                nc.tensor.matmul(
                                    ps2[:, 1:nW],
                                    lhsT=wfsb[:, cc, ti, :],
                                    rhs=xqs[q][cc][:, 1:126:2],
                                    start=False,
                                    stop=False,
                                )
                        for ci in range(6):
                            cc, dy = ci % 2, ci // 2
                            row_base = 2 * oh0 + dy - 1 - r0
                            l_lo = 1 if (top and dy in (0, 1)) else 0
                            l_cnt = 8 - l_lo
                            base = (row_base + 2 * l_lo) * 128 - 1
                            mm_rhs = xqs[q][cc][
                                :, base : base + (l_cnt - 1) * 256 + 1 : 256
                            ]
                            nc.tensor.matmul(
                                ps2[
                                    :,
                                    l_lo * nW : l_lo * nW
                                    + (l_cnt - 1) * nW
                                    + 1 : nW,
                                ],
                                lhsT=wnsb[:, cc, dy, :],
                                rhs=mm_rhs,
                                start=False,
                                stop=(ci == 5),
                            )
                        # bias field add -> bf16 logits
                        ker2b = smp.tile([KK, 512], BF16, tag="k2")
                        nc.vector.scalar_tensor_tensor(
                            out=ker2b[:],
                            in0=ps2[:],
                            scalar=1.0,
                            in1=bfsb[:, oh0 * nW : (oh0 + 8) * nW],
                            op0=mybir.AluOpType.mult,
                            op1=mybir.AluOpType.add,
                        )
                        # logit transposes -> [128, 4*26] PSUM (bf16, 4B-aligned)
                        pst = psT.tile([128, 4, KK + 1], BF16, tag="t")
                        for pp in range(4):
                            nc.tensor.transpose(
                                pst[:, pp, 0:KK],
                                ker2b[:, pp * 128 : (pp + 1) * 128],
                                idsb[:],
                            )
                        ecb = smp.tile([128, 4, KK], F32, tag="e")
                        nc.scalar.activation(
                            out=ecb[:],
                            in_=pst[:, :, 0:KK],
                            func=mybir.ActivationFunctionType.Exp,
                        )
                        scb = smp.tile([128, 4], F32, tag="s")
                        rcb = smp.tile([128, 4], F32, tag="r")
                        nc.vector.tensor_reduce(
                            out=scb[:],
                            in_=ecb[:],
                            axis=mybir.AxisListType.X,
                            op=mybir.AluOpType.add,
                        )
                        nc.vector.reciprocal(out=rcb[:], in_=scb[:])
                        for pp in range(4):
                            P = 8 * q + 4 * sub + pp
                            wn = smp.tile([128, NIDX], BF16, tag="wn", bufs=4)
                            nc.vector.tensor_scalar_mul(
                                out=wn[:, 0:KK],
                                in0=ecb[:, pp, :],
                                scalar1=rcb[:, pp : pp + 1],
                            )
                            nc.vector.memset(wn[:, KK:NIDX], 0.0)
                            at = atp.tile([128, QCH * 128], BF16, tag="at")
                            nc.gpsimd.local_scatter(
                                out_ap=at[:],
                                data_ap=wn[:],
                                idxs_ap=ixsb[:],
                                channels=128,
                                num_elems=QCH * 128,
                                num_idxs=NIDX,
                            )
                            q_ats.append((P, at))

                def emit_atpe():
                    # A matrices: PE-transpose each at into a PSUM bank,
                    # then one PSUM->SBUF copy per pair (DVE/Act alternate)
                    for P, at in pend_ats:
                        bankA = psA.tile([128, QCH, 128], BF16, tag="a")
                        for r in range(QCH):
                            nc.tensor.transpose(
                                bankA[:, r, :],
                                at[:, r * 128 : (r + 1) * 128],
                                id128[:],
                            )
                        amat = abp.tile([128, QCH, 128], BF16, tag="A")
                        if P % 2 == 0:
                            nc.vector.tensor_copy(
                                out=amat[:].rearrange("p a b -> p (a b)"),
                                in_=bankA[:].rearrange("p a b -> p (a b)"),
                            )
                        else:
                            nc.scalar.copy(
                                out=amat[:].rearrange("p a b -> p (a b)"),
                                in_=bankA[:].rearrange("p a b -> p (a b)"),
                            )
                        amats[P] = amat
                    pend_ats.clear()

                def emit_wsum(q):
                    osb0 = ostp.tile([128, 8 * 128], BF16, tag="o0", name=f"osb0_{q}")
                    osb1 = ostp.tile([128, 8 * 128], BF16, tag="o1", name=f"osb1_{q}")
                    osbs[q] = (osb0, osb1)
                    for sub in range(2):
                        for cc in range(2):
                            bank = psF.tile([128, 4, 128], F32, tag="f")
                            for pp in range(4):
                                P = 8 * q + 4 * sub + pp
                                qqs = [
                                    qq
                                    for qq in range(QCH)
                                    if 0 <= 4 * P - 2 + qq < H
                                ]
                                for qi, qq in enumerate(qqs):
                                    r = 4 * P - 2 + qq
                                    g = P // 8
                                    tl = g - 1 if r < 32 * g else g
                                    nc.tensor.matmul(
                                        bank[:, pp, :],
                                        lhsT=xTq[tl][
                                            :,
                                            r - 32 * tl,
                                            cc * 128 : (cc + 1) * 128,
                                        ],
                                        rhs=amats[P][:, qq, 0:128],
                                        start=(qi == 0),
                                        stop=(qi == len(qqs) - 1),
                                    )
                            osb = osb0 if cc == 0 else osb1
                            dst = osb[:, sub * 512 : (sub + 1) * 512]
                            src = bank[:].rearrange("p a b -> p (a b)")
                            if (2 * sub + cc) % 2 == 0:
                                nc.scalar.copy(out=dst, in_=src)
                            else:
                                nc.vector.tensor_copy(out=dst, in_=src)
                    nc.sync.dma_start(
                        out=o_d.ap()[0:128, q * 1024 : (q + 1) * 1024], in_=osb0[:]
                    )
                    nc.sync.dma_start(
                        out=o_d.ap()[128:256, q * 1024 : (q + 1) * 1024], in_=osb1[:]
                    )

                pend_ats = []
                for q in range(4):
                    emit_apath(q)
                    emit_atpe()
                    if q + 2 <= 3:
                        emit_load(q + 2)
                    if q + 1 <= 3:
                        emit_transpose(q + 1)
                    if q >= 1:
                        emit_wsum(q - 1)
                emit_wsum(3)

    nc.compile()
    return nc


_NC = None


def _get_nc():
    global _NC
    if _NC is None:
        _NC = _build_program()
    return _NC


def _host_inputs(w1, b1, w2, b2):
    """Precompute constant / rearranged weight tensors (host-side, numpy)."""
    bf = np.float16
    w1m = np.asarray(w1, np.float32).reshape(M, C)            # [m, c]
    w2m = np.asarray(w2, np.float32).reshape(KK, M, 9)        # [k, m, tau]
    # composed conv weights: weff[k, c, tau] = sum_m w2[k,m,tau] w1[m,c]
    weff_kct = np.einsum("kmt,mc->kct", w2m, w1m)             # [25, 256, 9]
    weff = np.transpose(
        weff_kct.reshape(KK, 2, 128, 9), (2, 1, 3, 0)
    ).astype(bf)                                              # [128, 2, 9, 25]
    weff = np.ascontiguousarray(weff)
    # negated dx=0 tap weights for the ow=0 column correction
    wneff = np.ascontiguousarray(-weff[:, :, [0, 3, 6], :])   # [128, 2, 3, 25]
    # per-pixel bias field: b2 + sum over VALID taps of (w2[:,:,tau] @ b1)
    b1v = np.asarray(b1, np.float32).reshape(M)
    b2v = np.asarray(b2, np.float32).reshape(KK)
    wb = np.einsum("kmt,m->kt", w2m, b1v)                     # [25, 9]
    oh = np.arange(nH)[:, None]
    ow = np.arange(nW)[None, :]
    beff = np.zeros((KK, nH, nW), np.float32)
    beff += b2v[:, None, None]
    for dy in range(3):
        for dx in range(3):
            valid = (
                (2 * oh + dy - 1 >= 0)
                & (2 * oh + dy - 1 < H)
                & (2 * ow + dx - 1 >= 0)
                & (2 * ow + dx - 1 < W)
            )
            beff += wb[:, 3 * dy + dx][:, None, None] * valid[None].astype(
                np.float32
            )
    beff = np.ascontiguousarray(beff.reshape(KK, NOPIX))
    ident = np.eye(KK, dtype=bf)
    ident128 = np.eye(128, dtype=bf)
    scidx = _exp_taps()
    return {
        "weff": weff,
        "wneff": wneff,
        "beff": beff,
        "ident": ident,
        "ident128": ident128,
        "scidx": scidx,
    }


def kernel(x, w1, b1, w2, b2):
    x = np.asarray(x, np.float32)
    consts = _host_inputs(w1, b1, w2, b2)
    nc = _get_nc()
    in_maps = []
    for b in range(B):
        m = {"x": np.ascontiguousarray(x[b].reshape(C, NPIX))}
        m.update(consts)
        in_maps.append(m)
    res = run_bass_kernel_spmd(nc, in_maps, core_ids=list(range(B)))
    out = np.stack([res.results[i]["out"] for i in range(B)], axis=0)
    return out.astype(np.float32).reshape(B, C, nH, nW)
